# revision 31
# baseline (speedup 1.0000x reference)
"""AMSoftmax (norm-free branch) Trainium2 kernel, 8 NeuronCores.

Reference computes, for input x [B,D], label [B], weight [C,D], scalars s,m:
    norm   = ||x||_2 per row                       [B,1]
    cosine = (x/max(norm,eps)) @ (w/max(||w||,eps)).T   [B,C]
    logits = norm * (cosine - m*onehot(label))     [B,C]
    returns (logits, cosine)

Key identity: norm * cosine == x @ w_hat.T exactly (norm/max(norm,eps) == 1
for any nonzero x), so per output element:
    raw    = x @ w_hat.T
    cosine = raw * (1/norm)            (per-row scale)
    logits = raw - norm*m*onehot       (one column per row)

Sharding: 2-way over batch x 4-way over classes (8 cores, no collectives;
outputs are disjoint tiles concatenated on host). Per core: x [2048,512],
w_hat [2000,512], outputs [2048,2000] each.

Schedule: W prep first (gates all matmuls), then X prep pipelined 4 row
tiles ahead of the matmul+epilogue+store loop so the output DMA stream
starts as early as possible and stays saturated.
"""

import os
import sys

sys.path.insert(0, "/opt/trn_rl_repo")

import numpy as np

B, D, C = 4096, 512, 8000
NB, NCL = 2, 4  # batch x class core grid
BL, CL = B // NB, C // NCL  # 2048, 2000 per core
RT = BL // 128  # 16 row tiles
KC = D // 128  # 4 contraction chunks
CW = 500  # matmul free-dim chunk (PSUM bank holds 512 f32)
HB = 2  # c chunks per half row tile
NH = CL // (CW * HB)  # 2 halves per row tile

COMPUTE = os.environ.get("AMS_DTYPE", "bf16")  # "bf16" or "f32r"
PREFETCH = 4

_CACHE = {}


def _build():
    import concourse.mybir as mybir
    import concourse.tile as tile
    from concourse import bacc
    from concourse.masks import make_identity

    f32 = mybir.dt.float32
    cdt = mybir.dt.bfloat16 if COMPUTE == "bf16" else mybir.dt.float32r
    tdt = mybir.dt.bfloat16 if COMPUTE == "bf16" else f32  # PE transpose psum dtype

    nc = bacc.Bacc()
    x_ext = nc.declare_dram_parameter("x", [BL, D], f32, isOutput=False)
    w_ext = nc.declare_dram_parameter("w", [CL, D], f32, isOutput=False)
    lab_ext = nc.declare_dram_parameter("lab", [128, RT], f32, isOutput=False)
    m_ext = nc.declare_dram_parameter("mvec", [128, 1], f32, isOutput=False)
    logits_ext = nc.declare_dram_parameter("logits", [BL, CL], f32, isOutput=True)
    cosine_ext = nc.declare_dram_parameter("cosine", [BL, CL], f32, isOutput=True)

    WT = (CL + 127) // 128  # 16 w row tiles (last one 80 partitions)

    with tile.TileContext(nc) as tc:
        with (
            tc.tile_pool(name="persist", bufs=1) as persist,
            tc.tile_pool(name="sq", bufs=2) as sq_pool,
            tc.tile_pool(name="sq4", bufs=1) as sq4_pool,
            tc.tile_pool(name="psT", bufs=2, space="PSUM") as psT_pool,
            tc.tile_pool(name="psM", bufs=3, space="PSUM") as psM_pool,
            tc.tile_pool(name="outb", bufs=3) as out_pool,
            tc.tile_pool(name="mm", bufs=3) as mm_pool,
        ):
            identity = persist.tile([128, 128], cdt)
            make_identity(nc, identity)
            identity_f32 = persist.tile([128, 128], f32)
            make_identity(nc, identity_f32)

            iota = persist.tile([128, CL], f32)
            nc.gpsimd.iota(
                iota[:],
                pattern=[[1, CL]],
                base=0,
                channel_multiplier=0,
                allow_small_or_imprecise_dtypes=True,
            )

            lab_sb = persist.tile([128, RT], f32)
            nc.sync.dma_start(lab_sb[:], lab_ext[:])
            m_sb = persist.tile([128, 1], f32)
            nc.sync.dma_start(m_sb[:], m_ext[:])

            w_in = persist.tile([128, WT, D], f32)
            nc.vector.memset(w_in[64:, WT - 1, :], 0.0)
            x_in = persist.tile([128, RT, D], f32)
            w_bf = persist.tile([128, WT, D], cdt)  # normalized W, compute dtype
            wts = []
            for k in range(KC):
                wt_k = persist.tile([128, CL], cdt, tag=f"wt{k}")
                wts.append(wt_k)
            xts = []
            for t in range(RT):
                xt_t = persist.tile([128, KC, 128], cdt, tag=f"xt{t}")
                xts.append(xt_t)

            xss = persist.tile([128, RT], f32)
            xnorm = persist.tile([128, RT], f32)
            inv_xnorm = persist.tile([128, RT], f32)
            norm_m = persist.tile([128, RT], f32)
            wss = persist.tile([128, WT], f32)
            inv_wnorm = persist.tile([128, WT], f32)
            # last W tile covers only 80 rows; pad so batched norm ops on
            # full partitions read defined data
            nc.vector.memset(wss[:], 1.0)

            # ---- batched input DMAs: W first (it gates the matmuls) ----
            for g in range(3):
                nc.sync.dma_start(
                    w_in[:, 4 * g : 4 * g + 4, :],
                    w_ext[512 * g : 512 * (g + 1), :].rearrange(
                        "(a p) d -> p a d", p=128
                    ),
                )
            nc.sync.dma_start(
                w_in[:, 12:15, :],
                w_ext[1536:1920, :].rearrange("(a p) d -> p a d", p=128),
            )
            nc.sync.dma_start(w_in[:80, 15, :], w_ext[1920:2000, :])
            for g in range(4):
                nc.sync.dma_start(
                    x_in[:, 4 * g : 4 * g + 4, :],
                    x_ext[512 * g : 512 * (g + 1), :].rearrange(
                        "(a p) d -> p a d", p=128
                    ),
                )

            # ---- W prep ----
            def w_sq_group(g):
                sq4 = sq4_pool.tile([128, 4, D], f32, tag="sq4")
                nc.scalar.activation(
                    sq4[:],
                    w_in[:, 4 * g : 4 * g + 4, :],
                    mybir.ActivationFunctionType.Square,
                )
                nc.vector.reduce_sum(
                    wss[:, 4 * g : 4 * g + 4], sq4[:], axis=mybir.AxisListType.X
                )

            def w_norms(g):
                cs = slice(4 * g, 4 * g + 4)
                nc.scalar.sqrt(wss[:, cs], wss[:, cs])
                nc.vector.tensor_scalar_max(wss[:, cs], wss[:, cs], 1e-12)
                nc.vector.reciprocal(inv_wnorm[:, cs], wss[:, cs])

            def w_cast(t):
                # fused normalize + cast, alternating engines
                p = min(128, CL - t * 128)
                if t % 2 == 0:
                    nc.scalar.mul(
                        w_bf[:p, t, :], w_in[:p, t, :], inv_wnorm[:p, t : t + 1]
                    )
                else:
                    nc.vector.tensor_scalar_mul(
                        w_bf[:p, t, :], w_in[:p, t, :], inv_wnorm[:p, t : t + 1]
                    )

            def w_transpose(k, g):
                # transpose the k-th 128-d chunk of W tiles 4g..4g+3 into wts[k]
                ps = psT_pool.tile([128, 4, 128], tdt, tag="psT")
                for j in range(4):
                    t = 4 * g + j
                    p = min(128, CL - t * 128)
                    nc.tensor.transpose(
                        ps[:, j, :p],
                        w_bf[:p, t, k * 128 : (k + 1) * 128],
                        identity[:p, :p],
                    )
                eng = nc.vector.tensor_copy if g % 2 == 0 else nc.scalar.copy
                if g < 3:
                    eng(wts[k][:, 512 * g : 512 * (g + 1)], ps[:])
                else:
                    eng(wts[k][:, 1536:1920], ps[:, :3, :])
                    eng(wts[k][:, 1920:2000], ps[:, 3, :80])

            # ---- X prep: row sumsq (ACT), cast (DVE), transpose (PE) ----
            def x_prep(t):
                sq = sq_pool.tile([128, D], f32, tag="sq")
                if t % 2 == 0:
                    nc.scalar.activation(
                        sq[:],
                        x_in[:, t, :],
                        mybir.ActivationFunctionType.Square,
                        accum_out=xss[:, t : t + 1],
                    )
                else:
                    nc.vector.tensor_mul(sq[:], x_in[:, t, :], x_in[:, t, :])
                    nc.vector.reduce_sum(
                        xss[:, t : t + 1], sq[:], axis=mybir.AxisListType.X
                    )
                ps = psT_pool.tile([128, KC, 128], f32, tag="psT")
                for k in range(KC):
                    nc.tensor.transpose(
                        ps[:, k, :],
                        x_in[:, t, k * 128 : (k + 1) * 128],
                        identity_f32[:],
                    )
                # fused f32 -> compute-dtype cast in the PSUM drain
                if t % 2 == 0:
                    nc.scalar.copy(xts[t][:], ps[:])
                else:
                    nc.vector.tensor_copy(xts[t][:], ps[:])

            def x_norms(g):
                cs = slice(2 * g, 2 * g + 2)
                nc.scalar.sqrt(xnorm[:, cs], xss[:, cs])
                nc.vector.tensor_scalar_max(xnorm[:, cs], xnorm[:, cs], 1e-12)
                nc.vector.reciprocal(inv_xnorm[:, cs], xnorm[:, cs])
                nc.vector.tensor_mul(
                    norm_m[:, cs], xnorm[:, cs], m_sb.broadcast_to([128, 2])
                )

            # ---- main loop body: matmul + epilogue + store for row tile t ----
            masks = {}

            def mask(t):
                mmt = mm_pool.tile([128, CL], f32, tag="mm")
                nc.vector.tensor_scalar(
                    mmt[:],
                    iota[:],
                    scalar1=lab_sb[:, t : t + 1],
                    scalar2=norm_m[:, t : t + 1],
                    op0=mybir.AluOpType.is_equal,
                    op1=mybir.AluOpType.mult,
                )
                masks[t] = mmt

            def main(t):
                logits_sb = out_pool.tile([128, CL], f32, tag="logits")
                cosine_sb = out_pool.tile([128, CL], f32, tag="cosine")
                mmt = masks.pop(t)
                for h in range(NH):
                    ps = psM_pool.tile([128, HB, 512], f32, tag="psM")
                    for k in range(KC):
                        for cc in range(HB):
                            c0 = (h * HB + cc) * CW
                            nc.tensor.matmul(
                                ps[:, cc, :CW],
                                xts[t][:, k, :],
                                wts[k][:, c0 : c0 + CW],
                                start=(k == 0),
                                stop=(k == KC - 1),
                            )
                    ps3 = ps[:, :, :CW]
                    hs = slice(h * HB * CW, (h + 1) * HB * CW)
                    nc.scalar.activation(
                        cosine_sb[:, hs].rearrange("p (a b) -> p a b", a=HB),
                        ps3,
                        mybir.ActivationFunctionType.Copy,
                        scale=inv_xnorm[:, t : t + 1],
                    )
                    nc.vector.tensor_sub(
                        logits_sb[:, hs].rearrange("p (a b) -> p a b", a=HB),
                        ps3,
                        mmt[:, hs].rearrange("p (a b) -> p a b", a=HB),
                    )
                nc.sync.dma_start(
                    logits_ext[t * 128 : (t + 1) * 128, :], logits_sb[:]
                )
                nc.sync.dma_start(
                    cosine_ext[t * 128 : (t + 1) * 128, :], cosine_sb[:]
                )


            # ---- emission: W chain, early X, then the pipelined main loop
            for g in range(4):
                w_sq_group(g)
                w_norms(g)
                for t in range(4 * g, 4 * g + 4):
                    w_cast(t)
            for k in range(KC):
                for g in range(4):
                    w_transpose(k, g)
            x_prep(0)
            x_prep(1)
            x_norms(0)
            for t in range(RT):
                nt = t + 2
                if nt < RT:
                    x_prep(nt)
                    if nt % 2 == 1:
                        x_norms(nt // 2)
                mask(t)
                main(t)

    nc.finalize()
    return nc


def _in_maps(x, w, lab, mval):
    maps = []
    for ci in range(8):
        bi, cj = ci // NCL, ci % NCL
        b0, c0 = bi * BL, cj * CL
        lab_local = (lab[b0 : b0 + BL] - c0).reshape(RT, 128).T
        maps.append(
            {
                "x": x[b0 : b0 + BL],
                "w": w[c0 : c0 + CL],
                "lab": np.ascontiguousarray(lab_local),
                "mvec": np.full((128, 1), mval, dtype=np.float32),
            }
        )
    return maps


def kernel(input, label, weight, s, m):
    from concourse.bass_utils import run_bass_kernel_spmd

    if "nc" not in _CACHE:
        _CACHE["nc"] = _build()
    nc = _CACHE["nc"]

    x = np.ascontiguousarray(np.asarray(input, dtype=np.float32))
    w = np.ascontiguousarray(np.asarray(weight, dtype=np.float32))
    lab = np.asarray(label).astype(np.float32)
    mval = float(np.asarray(m))

    res = run_bass_kernel_spmd(nc, _in_maps(x, w, lab, mval), core_ids=list(range(8)))

    logits = np.empty((B, C), dtype=np.float32)
    cosine = np.empty((B, C), dtype=np.float32)
    for ci in range(8):
        bi, cj = ci // NCL, ci % NCL
        b0, c0 = bi * BL, cj * CL
        logits[b0 : b0 + BL, c0 : c0 + CL] = res.results[ci]["logits"]
        cosine[b0 : b0 + BL, c0 : c0 + CL] = res.results[ci]["cosine"]
    return logits, cosine


# revision 32
# speedup vs baseline: 1.0405x; 1.0405x over previous
"""AMSoftmax (norm-free branch) Trainium2 kernel, 8 NeuronCores.

Reference computes, for input x [B,D], label [B], weight [C,D], scalars s,m:
    norm   = ||x||_2 per row                       [B,1]
    cosine = (x/max(norm,eps)) @ (w/max(||w||,eps)).T   [B,C]
    logits = norm * (cosine - m*onehot(label))     [B,C]
    returns (logits, cosine)

Key identity: norm * cosine == x @ w_hat.T exactly (norm/max(norm,eps) == 1
for any nonzero x), so per output element:
    raw    = x @ w_hat.T
    cosine = raw * (1/norm)            (per-row scale)
    logits = raw - norm*m*onehot       (one column per row)

Sharding: 2-way over batch x 4-way over classes (8 cores, no collectives;
outputs are disjoint tiles concatenated on host). Per core: x [2048,512],
w_hat [2000,512], outputs [2048,2000] each.

Schedule: W prep first (gates all matmuls), then X prep pipelined 4 row
tiles ahead of the matmul+epilogue+store loop so the output DMA stream
starts as early as possible and stays saturated.
"""

import os
import sys

sys.path.insert(0, "/opt/trn_rl_repo")

import numpy as np

B, D, C = 4096, 512, 8000
NB, NCL = 2, 4  # batch x class core grid
BL, CL = B // NB, C // NCL  # 2048, 2000 per core
RT = BL // 128  # 16 row tiles
KC = D // 128  # 4 contraction chunks
CW = 500  # matmul free-dim chunk (PSUM bank holds 512 f32)
HB = 2  # c chunks per half row tile
NH = CL // (CW * HB)  # 2 halves per row tile

COMPUTE = os.environ.get("AMS_DTYPE", "bf16")  # "bf16" or "f32r"
PREFETCH = 4

_CACHE = {}


def _build():
    import concourse.mybir as mybir
    import concourse.tile as tile
    from concourse import bacc
    from concourse.masks import make_identity

    f32 = mybir.dt.float32
    cdt = mybir.dt.bfloat16 if COMPUTE == "bf16" else mybir.dt.float32r
    tdt = mybir.dt.bfloat16 if COMPUTE == "bf16" else f32  # PE transpose psum dtype

    nc = bacc.Bacc()
    x_ext = nc.declare_dram_parameter("x", [BL, D], f32, isOutput=False)
    w_ext = nc.declare_dram_parameter("w", [CL, D], f32, isOutput=False)
    lab_ext = nc.declare_dram_parameter("lab", [128, RT], f32, isOutput=False)
    m_ext = nc.declare_dram_parameter("mvec", [128, 1], f32, isOutput=False)
    logits_ext = nc.declare_dram_parameter("logits", [BL, CL], f32, isOutput=True)
    cosine_ext = nc.declare_dram_parameter("cosine", [BL, CL], f32, isOutput=True)

    WT = (CL + 127) // 128  # 16 w row tiles (last one 80 partitions)

    with tile.TileContext(nc) as tc:
        with (
            tc.tile_pool(name="persist", bufs=1) as persist,
            tc.tile_pool(name="sq", bufs=2) as sq_pool,
            tc.tile_pool(name="sq4", bufs=1) as sq4_pool,
            tc.tile_pool(name="psT", bufs=2, space="PSUM") as psT_pool,
            tc.tile_pool(name="psM", bufs=3, space="PSUM") as psM_pool,
            tc.tile_pool(name="outb", bufs=3) as out_pool,
            tc.tile_pool(name="mm", bufs=3) as mm_pool,
        ):
            identity = persist.tile([128, 128], cdt)
            make_identity(nc, identity)
            identity_f32 = persist.tile([128, 128], f32)
            make_identity(nc, identity_f32)

            iota = persist.tile([128, CL], f32)
            nc.gpsimd.iota(
                iota[:],
                pattern=[[1, CL]],
                base=0,
                channel_multiplier=0,
                allow_small_or_imprecise_dtypes=True,
            )

            lab_sb = persist.tile([128, RT], f32)
            nc.sync.dma_start(lab_sb[:], lab_ext[:])
            m_sb = persist.tile([128, 1], f32)
            nc.sync.dma_start(m_sb[:], m_ext[:])

            w_in = persist.tile([128, WT, D], f32)
            nc.vector.memset(w_in[64:, WT - 1, :], 0.0)
            x_in = persist.tile([128, RT, D], f32)
            w_bf = persist.tile([128, WT, D], cdt)  # normalized W, compute dtype
            wtsA, wtsB = [], []
            for k in range(KC):
                wt_ka = persist.tile([128, 1024], cdt, tag=f"wtA{k}")
                wtsA.append(wt_ka)
            for k in range(KC):
                wt_kb = persist.tile([128, CL - 1024], cdt, tag=f"wtB{k}")
                wtsB.append(wt_kb)
            xts = []
            for t in range(RT):
                xt_t = persist.tile([128, KC, 128], cdt, tag=f"xt{t}")
                xts.append(xt_t)

            xss = persist.tile([128, RT], f32)
            xnorm = persist.tile([128, RT], f32)
            inv_xnorm = persist.tile([128, RT], f32)
            norm_m = persist.tile([128, RT], f32)
            wss = persist.tile([128, WT], f32)
            inv_wnorm = persist.tile([128, WT], f32)
            # last W tile covers only 80 rows; pad so batched norm ops on
            # full partitions read defined data
            nc.vector.memset(wss[:], 1.0)

            # ---- batched input DMAs: W first (it gates the matmuls) ----
            for g in range(3):
                nc.sync.dma_start(
                    w_in[:, 4 * g : 4 * g + 4, :],
                    w_ext[512 * g : 512 * (g + 1), :].rearrange(
                        "(a p) d -> p a d", p=128
                    ),
                )
            nc.sync.dma_start(
                w_in[:, 12:15, :],
                w_ext[1536:1920, :].rearrange("(a p) d -> p a d", p=128),
            )
            nc.sync.dma_start(w_in[:80, 15, :], w_ext[1920:2000, :])
            for g in range(4):
                nc.sync.dma_start(
                    x_in[:, 4 * g : 4 * g + 4, :],
                    x_ext[512 * g : 512 * (g + 1), :].rearrange(
                        "(a p) d -> p a d", p=128
                    ),
                )

            # ---- W prep ----
            def w_sq_group(g):
                sq4 = sq4_pool.tile([128, 4, D], f32, tag="sq4")
                nc.scalar.activation(
                    sq4[:],
                    w_in[:, 4 * g : 4 * g + 4, :],
                    mybir.ActivationFunctionType.Square,
                )
                nc.vector.reduce_sum(
                    wss[:, 4 * g : 4 * g + 4], sq4[:], axis=mybir.AxisListType.X
                )

            def w_norms(g):
                cs = slice(4 * g, 4 * g + 4)
                nc.scalar.sqrt(wss[:, cs], wss[:, cs])
                nc.vector.tensor_scalar_max(wss[:, cs], wss[:, cs], 1e-12)
                nc.vector.reciprocal(inv_wnorm[:, cs], wss[:, cs])

            def w_cast(t):
                # fused normalize + cast, alternating engines
                p = min(128, CL - t * 128)
                if t % 2 == 0:
                    nc.scalar.mul(
                        w_bf[:p, t, :], w_in[:p, t, :], inv_wnorm[:p, t : t + 1]
                    )
                else:
                    nc.vector.tensor_scalar_mul(
                        w_bf[:p, t, :], w_in[:p, t, :], inv_wnorm[:p, t : t + 1]
                    )

            def w_transpose(k, g):
                # transpose the k-th 128-d chunk of W tiles 4g..4g+3 into wts[k]
                ps = psT_pool.tile([128, 4, 128], tdt, tag="psT")
                for j in range(4):
                    t = 4 * g + j
                    p = min(128, CL - t * 128)
                    nc.tensor.transpose(
                        ps[:, j, :p],
                        w_bf[:p, t, k * 128 : (k + 1) * 128],
                        identity[:p, :p],
                    )
                eng = nc.vector.tensor_copy if g % 2 == 0 else nc.scalar.copy
                if g < 2:
                    eng(wtsA[k][:, 512 * g : 512 * (g + 1)], ps[:])
                elif g == 2:
                    eng(wtsB[k][:, 0:512], ps[:])
                else:
                    eng(wtsB[k][:, 512:896], ps[:, :3, :])
                    eng(wtsB[k][:, 896:976], ps[:, 3, :80])

            # ---- X prep: row sumsq (ACT), cast (DVE), transpose (PE) ----
            def x_prep(t):
                sq = sq_pool.tile([128, D], f32, tag="sq")
                if t % 2 == 0:
                    nc.scalar.activation(
                        sq[:],
                        x_in[:, t, :],
                        mybir.ActivationFunctionType.Square,
                        accum_out=xss[:, t : t + 1],
                    )
                else:
                    nc.vector.tensor_mul(sq[:], x_in[:, t, :], x_in[:, t, :])
                    nc.vector.reduce_sum(
                        xss[:, t : t + 1], sq[:], axis=mybir.AxisListType.X
                    )
                ps = psT_pool.tile([128, KC, 128], f32, tag="psT")
                for k in range(KC):
                    nc.tensor.transpose(
                        ps[:, k, :],
                        x_in[:, t, k * 128 : (k + 1) * 128],
                        identity_f32[:],
                    )
                # fused f32 -> compute-dtype cast in the PSUM drain
                if t % 2 == 0:
                    nc.scalar.copy(xts[t][:], ps[:])
                else:
                    nc.vector.tensor_copy(xts[t][:], ps[:])

            def x_norms(g):
                cs = slice(2 * g, 2 * g + 2)
                nc.scalar.sqrt(xnorm[:, cs], xss[:, cs])
                nc.vector.tensor_scalar_max(xnorm[:, cs], xnorm[:, cs], 1e-12)
                nc.vector.reciprocal(inv_xnorm[:, cs], xnorm[:, cs])
                nc.vector.tensor_mul(
                    norm_m[:, cs], xnorm[:, cs], m_sb.broadcast_to([128, 2])
                )

            # ---- main loop body: matmul + epilogue + store for row tile t ----
            masks = {}

            def mask(t):
                mmt = mm_pool.tile([128, CL], f32, tag="mm")
                nc.vector.tensor_scalar(
                    mmt[:],
                    iota[:],
                    scalar1=lab_sb[:, t : t + 1],
                    scalar2=norm_m[:, t : t + 1],
                    op0=mybir.AluOpType.is_equal,
                    op1=mybir.AluOpType.mult,
                )
                masks[t] = mmt

            def main(t):
                logits_sb = out_pool.tile([128, CL], f32, tag="logits")
                cosine_sb = out_pool.tile([128, CL], f32, tag="cosine")
                mmt = masks.pop(t)
                for h in range(NH):
                    ps = psM_pool.tile([128, HB, 512], f32, tag="psM")
                    src_half = wtsA if h == 0 else wtsB
                    for k in range(KC):
                        for cc in range(HB):
                            cw = 512 if (h == 0 or cc == 0) else 464
                            nc.tensor.matmul(
                                ps[:, cc, :cw],
                                xts[t][:, k, :],
                                src_half[k][:, 512 * cc : 512 * cc + cw],
                                start=(k == 0),
                                stop=(k == KC - 1),
                            )
                    if h == 0:
                        nc.scalar.activation(
                            cosine_sb[:, :1024].rearrange(
                                "p (a b) -> p a b", a=HB
                            ),
                            ps[:],
                            mybir.ActivationFunctionType.Copy,
                            scale=inv_xnorm[:, t : t + 1],
                        )
                        nc.vector.tensor_sub(
                            logits_sb[:, :1024].rearrange(
                                "p (a b) -> p a b", a=HB
                            ),
                            ps[:],
                            mmt[:, :1024].rearrange("p (a b) -> p a b", a=HB),
                        )
                        hs = slice(0, 1024)
                    else:
                        nc.scalar.activation(
                            cosine_sb[:, 1024:2000].rearrange(
                                "p (a b) -> p a b", b=488
                            ),
                            ps[:].rearrange("p a b -> p (a b)")[:, :976].rearrange(
                                "p (a b) -> p a b", b=488
                            ),
                            mybir.ActivationFunctionType.Copy,
                            scale=inv_xnorm[:, t : t + 1],
                        )
                        nc.vector.tensor_sub(
                            logits_sb[:, 1024:2000],
                            ps[:].rearrange("p a b -> p (a b)")[:, :976],
                            mmt[:, 1024:2000],
                        )
                        hs = slice(1024, 2000)
                    nc.sync.dma_start(
                        logits_ext[t * 128 : (t + 1) * 128, hs], logits_sb[:, hs]
                    )
                    nc.sync.dma_start(
                        cosine_ext[t * 128 : (t + 1) * 128, hs], cosine_sb[:, hs]
                    )


            # ---- emission: W chain, early X, then the pipelined main loop
            for g in range(4):
                w_sq_group(g)
                w_norms(g)
                for t in range(4 * g, 4 * g + 4):
                    w_cast(t)
            for k in range(KC):
                for g in range(4):
                    w_transpose(k, g)
            x_prep(0)
            x_prep(1)
            x_norms(0)
            for t in range(RT):
                nt = t + 2
                if nt < RT:
                    x_prep(nt)
                    if nt % 2 == 1:
                        x_norms(nt // 2)
                mask(t)
                main(t)

    nc.finalize()
    return nc


def _in_maps(x, w, lab, mval):
    maps = []
    for ci in range(8):
        bi, cj = ci // NCL, ci % NCL
        b0, c0 = bi * BL, cj * CL
        lab_local = (lab[b0 : b0 + BL] - c0).reshape(RT, 128).T
        maps.append(
            {
                "x": x[b0 : b0 + BL],
                "w": w[c0 : c0 + CL],
                "lab": np.ascontiguousarray(lab_local),
                "mvec": np.full((128, 1), mval, dtype=np.float32),
            }
        )
    return maps


def kernel(input, label, weight, s, m):
    from concourse.bass_utils import run_bass_kernel_spmd

    if "nc" not in _CACHE:
        _CACHE["nc"] = _build()
    nc = _CACHE["nc"]

    x = np.ascontiguousarray(np.asarray(input, dtype=np.float32))
    w = np.ascontiguousarray(np.asarray(weight, dtype=np.float32))
    lab = np.asarray(label).astype(np.float32)
    mval = float(np.asarray(m))

    res = run_bass_kernel_spmd(nc, _in_maps(x, w, lab, mval), core_ids=list(range(8)))

    logits = np.empty((B, C), dtype=np.float32)
    cosine = np.empty((B, C), dtype=np.float32)
    for ci in range(8):
        bi, cj = ci // NCL, ci % NCL
        b0, c0 = bi * BL, cj * CL
        logits[b0 : b0 + BL, c0 : c0 + CL] = res.results[ci]["logits"]
        cosine[b0 : b0 + BL, c0 : c0 + CL] = res.results[ci]["cosine"]
    return logits, cosine


# revision 34
# speedup vs baseline: 1.0602x; 1.0190x over previous
"""AMSoftmax (norm-free branch) Trainium2 kernel, 8 NeuronCores.

Reference computes, for input x [B,D], label [B], weight [C,D], scalars s,m:
    norm   = ||x||_2 per row                       [B,1]
    cosine = (x/max(norm,eps)) @ (w/max(||w||,eps)).T   [B,C]
    logits = norm * (cosine - m*onehot(label))     [B,C]
    returns (logits, cosine)

Key identity: norm * cosine == x @ w_hat.T exactly (norm/max(norm,eps) == 1
for any nonzero x), so per output element:
    raw    = x @ w_hat.T
    cosine = raw * (1/norm)            (per-row scale)
    logits = raw - norm*m*onehot       (one column per row)

Sharding: 2-way over batch x 4-way over classes (8 cores, no collectives;
outputs are disjoint tiles concatenated on host). Per core: x [2048,512],
w_hat [2000,512], outputs [2048,2000] each.

Schedule: W prep first (gates all matmuls), then X prep pipelined 4 row
tiles ahead of the matmul+epilogue+store loop so the output DMA stream
starts as early as possible and stays saturated.
"""

import os
import sys

sys.path.insert(0, "/opt/trn_rl_repo")

import numpy as np

B, D, C = 4096, 512, 8000
NB, NCL = 2, 4  # batch x class core grid
BL, CL = B // NB, C // NCL  # 2048, 2000 per core
RT = BL // 128  # 16 row tiles
KC = D // 128  # 4 contraction chunks
CW = 500  # matmul free-dim chunk (PSUM bank holds 512 f32)
HB = 2  # c chunks per half row tile
NH = CL // (CW * HB)  # 2 halves per row tile

COMPUTE = os.environ.get("AMS_DTYPE", "bf16")  # "bf16" or "f32r"
PREFETCH = 4

_CACHE = {}


def _build():
    import concourse.mybir as mybir
    import concourse.tile as tile
    from concourse import bacc
    from concourse.masks import make_identity

    f32 = mybir.dt.float32
    cdt = mybir.dt.bfloat16 if COMPUTE == "bf16" else mybir.dt.float32r
    tdt = mybir.dt.bfloat16 if COMPUTE == "bf16" else f32  # PE transpose psum dtype

    nc = bacc.Bacc()
    x_ext = nc.declare_dram_parameter("x", [BL, D], f32, isOutput=False)
    w_ext = nc.declare_dram_parameter("w", [CL, D], f32, isOutput=False)
    lab_ext = nc.declare_dram_parameter("lab", [128, RT], f32, isOutput=False)
    m_ext = nc.declare_dram_parameter("mvec", [128, 1], f32, isOutput=False)
    logits_ext = nc.declare_dram_parameter("logits", [BL, CL], f32, isOutput=True)
    cosine_ext = nc.declare_dram_parameter("cosine", [BL, CL], f32, isOutput=True)

    WT = (CL + 127) // 128  # 16 w row tiles (last one 80 partitions)

    with tile.TileContext(nc) as tc:
        with (
            tc.tile_pool(name="persist", bufs=1) as persist,
            tc.tile_pool(name="sq", bufs=2) as sq_pool,
            tc.tile_pool(name="sq4", bufs=1) as sq4_pool,
            tc.tile_pool(name="psT", bufs=2, space="PSUM") as psT_pool,
            tc.tile_pool(name="psM", bufs=3, space="PSUM") as psM_pool,
            tc.tile_pool(name="outb", bufs=3) as out_pool,
            tc.tile_pool(name="mm", bufs=3) as mm_pool,
        ):
            identity = persist.tile([128, 128], cdt)
            make_identity(nc, identity)
            identity_f32 = persist.tile([128, 128], f32)
            make_identity(nc, identity_f32)

            iota = persist.tile([128, CL], f32)
            nc.gpsimd.iota(
                iota[:],
                pattern=[[1, CL]],
                base=0,
                channel_multiplier=0,
                allow_small_or_imprecise_dtypes=True,
            )

            lab_sb = persist.tile([128, RT], f32)
            nc.sync.dma_start(lab_sb[:], lab_ext[:])
            m_sb = persist.tile([128, 1], f32)
            nc.sync.dma_start(m_sb[:], m_ext[:])

            w_in = persist.tile([128, WT, D], f32)
            nc.vector.memset(w_in[64:, WT - 1, :], 0.0)
            x_in = persist.tile([128, RT, D], f32)
            w_bf = persist.tile([128, WT, D], cdt)  # normalized W, compute dtype
            wts = []
            for k in range(KC):
                wt_k = persist.tile([128, CL], cdt, tag=f"wt{k}")
                wts.append(wt_k)
            xts = []
            for t in range(RT):
                xt_t = persist.tile([128, KC, 128], cdt, tag=f"xt{t}")
                xts.append(xt_t)

            xss = persist.tile([128, RT], f32)
            xnorm = persist.tile([128, RT], f32)
            inv_xnorm = persist.tile([128, RT], f32)
            norm_m = persist.tile([128, RT], f32)
            wss = persist.tile([128, WT], f32)
            inv_wnorm = persist.tile([128, WT], f32)
            # last W tile covers only 80 rows; pad so batched norm ops on
            # full partitions read defined data
            nc.vector.memset(wss[:], 1.0)

            # ---- batched input DMAs: W first (it gates the matmuls) ----
            for g in range(3):
                nc.sync.dma_start(
                    w_in[:, 4 * g : 4 * g + 4, :],
                    w_ext[512 * g : 512 * (g + 1), :].rearrange(
                        "(a p) d -> p a d", p=128
                    ),
                )
            nc.sync.dma_start(
                w_in[:, 12:15, :],
                w_ext[1536:1920, :].rearrange("(a p) d -> p a d", p=128),
            )
            nc.sync.dma_start(w_in[:80, 15, :], w_ext[1920:2000, :])
            for g in range(4):
                nc.sync.dma_start(
                    x_in[:, 4 * g : 4 * g + 4, :],
                    x_ext[512 * g : 512 * (g + 1), :].rearrange(
                        "(a p) d -> p a d", p=128
                    ),
                )

            # ---- W prep ----
            def w_sq_group(g):
                sq4 = sq4_pool.tile([128, 4, D], f32, tag="sq4")
                nc.scalar.activation(
                    sq4[:],
                    w_in[:, 4 * g : 4 * g + 4, :],
                    mybir.ActivationFunctionType.Square,
                )
                nc.vector.reduce_sum(
                    wss[:, 4 * g : 4 * g + 4], sq4[:], axis=mybir.AxisListType.X
                )

            def w_norms(g):
                cs = slice(4 * g, 4 * g + 4)
                nc.scalar.sqrt(wss[:, cs], wss[:, cs])
                nc.vector.tensor_scalar_max(wss[:, cs], wss[:, cs], 1e-12)
                nc.vector.reciprocal(inv_wnorm[:, cs], wss[:, cs])

            def w_cast(t):
                # fused normalize + cast, alternating engines
                p = min(128, CL - t * 128)
                if t % 2 == 0:
                    nc.scalar.mul(
                        w_bf[:p, t, :], w_in[:p, t, :], inv_wnorm[:p, t : t + 1]
                    )
                else:
                    nc.vector.tensor_scalar_mul(
                        w_bf[:p, t, :], w_in[:p, t, :], inv_wnorm[:p, t : t + 1]
                    )

            def w_transpose(k, g):
                # transpose the k-th 128-d chunk of W tiles 4g..4g+3 into wts[k]
                ps = psT_pool.tile([128, 4, 128], tdt, tag="psT")
                for j in range(4):
                    t = 4 * g + j
                    p = min(128, CL - t * 128)
                    nc.tensor.transpose(
                        ps[:, j, :p],
                        w_bf[:p, t, k * 128 : (k + 1) * 128],
                        identity[:p, :p],
                    )
                eng = nc.vector.tensor_copy if g % 2 == 0 else nc.scalar.copy
                if g < 3:
                    eng(wts[k][:, 512 * g : 512 * (g + 1)], ps[:])
                else:
                    eng(wts[k][:, 1536:1920], ps[:, :3, :])
                    eng(wts[k][:, 1920:2000], ps[:, 3, :80])

            # ---- X prep: row sumsq (ACT), cast (DVE), transpose (PE) ----
            def x_prep(t):
                sq = sq_pool.tile([128, D], f32, tag="sq")
                if t % 2 == 0:
                    nc.scalar.activation(
                        sq[:],
                        x_in[:, t, :],
                        mybir.ActivationFunctionType.Square,
                        accum_out=xss[:, t : t + 1],
                    )
                else:
                    nc.vector.tensor_mul(sq[:], x_in[:, t, :], x_in[:, t, :])
                    nc.vector.reduce_sum(
                        xss[:, t : t + 1], sq[:], axis=mybir.AxisListType.X
                    )
                ps = psT_pool.tile([128, KC, 128], f32, tag="psT")
                for k in range(KC):
                    nc.tensor.transpose(
                        ps[:, k, :],
                        x_in[:, t, k * 128 : (k + 1) * 128],
                        identity_f32[:],
                    )
                # fused f32 -> compute-dtype cast in the PSUM drain
                if t % 2 == 0:
                    nc.scalar.copy(xts[t][:], ps[:])
                else:
                    nc.vector.tensor_copy(xts[t][:], ps[:])

            def x_norms(g):
                cs = slice(2 * g, 2 * g + 2)
                nc.scalar.sqrt(xnorm[:, cs], xss[:, cs])
                nc.vector.tensor_scalar_max(xnorm[:, cs], xnorm[:, cs], 1e-12)
                nc.vector.reciprocal(inv_xnorm[:, cs], xnorm[:, cs])
                nc.vector.tensor_mul(
                    norm_m[:, cs], xnorm[:, cs], m_sb.broadcast_to([128, 2])
                )

            # ---- main loop body: matmul + epilogue + store for row tile t ----
            masks = {}

            def mask(t):
                mmt = mm_pool.tile([128, CL], f32, tag="mm")
                nc.vector.tensor_scalar(
                    mmt[:],
                    iota[:],
                    scalar1=lab_sb[:, t : t + 1],
                    scalar2=norm_m[:, t : t + 1],
                    op0=mybir.AluOpType.is_equal,
                    op1=mybir.AluOpType.mult,
                )
                masks[t] = mmt

            def main(t):
                logits_sb = out_pool.tile([128, CL], f32, tag="logits")
                cosine_sb = out_pool.tile([128, CL], f32, tag="cosine")
                mmt = masks.pop(t)
                for h in range(NH):
                    ps = psM_pool.tile([128, HB, 512], f32, tag="psM")
                    for k in range(KC):
                        for cc in range(HB):
                            c0 = (h * HB + cc) * CW
                            nc.tensor.matmul(
                                ps[:, cc, :CW],
                                xts[t][:, k, :],
                                wts[k][:, c0 : c0 + CW],
                                start=(k == 0),
                                stop=(k == KC - 1),
                            )
                    ps3 = ps[:, :, :CW]
                    hs = slice(h * HB * CW, (h + 1) * HB * CW)
                    nc.scalar.activation(
                        cosine_sb[:, hs].rearrange("p (a b) -> p a b", a=HB),
                        ps3,
                        mybir.ActivationFunctionType.Copy,
                        scale=inv_xnorm[:, t : t + 1],
                    )
                    nc.vector.tensor_sub(
                        logits_sb[:, hs].rearrange("p (a b) -> p a b", a=HB),
                        ps3,
                        mmt[:, hs].rearrange("p (a b) -> p a b", a=HB),
                    )
                nc.sync.dma_start(
                    cosine_ext[t * 128 : (t + 1) * 128, :], cosine_sb[:]
                )
                nc.sync.dma_start(
                    logits_ext[t * 128 : (t + 1) * 128, :], logits_sb[:]
                )


            # ---- emission: W chain, early X, then the pipelined main loop
            for g in range(4):
                w_sq_group(g)
                w_norms(g)
                for t in range(4 * g, 4 * g + 4):
                    w_cast(t)
            for k in range(KC):
                for g in range(4):
                    w_transpose(k, g)
            x_prep(0)
            x_prep(1)
            x_norms(0)
            mask(0)
            for t in range(RT):
                nt = t + 2
                if nt < RT:
                    x_prep(nt)
                    if nt % 2 == 1:
                        x_norms(nt // 2)
                if t + 1 < RT:
                    mask(t + 1)
                main(t)

    nc.finalize()
    return nc


def _in_maps(x, w, lab, mval):
    maps = []
    for ci in range(8):
        bi, cj = ci // NCL, ci % NCL
        b0, c0 = bi * BL, cj * CL
        lab_local = (lab[b0 : b0 + BL] - c0).reshape(RT, 128).T
        maps.append(
            {
                "x": x[b0 : b0 + BL],
                "w": w[c0 : c0 + CL],
                "lab": np.ascontiguousarray(lab_local),
                "mvec": np.full((128, 1), mval, dtype=np.float32),
            }
        )
    return maps


def kernel(input, label, weight, s, m):
    from concourse.bass_utils import run_bass_kernel_spmd

    if "nc" not in _CACHE:
        _CACHE["nc"] = _build()
    nc = _CACHE["nc"]

    x = np.ascontiguousarray(np.asarray(input, dtype=np.float32))
    w = np.ascontiguousarray(np.asarray(weight, dtype=np.float32))
    lab = np.asarray(label).astype(np.float32)
    mval = float(np.asarray(m))

    res = run_bass_kernel_spmd(nc, _in_maps(x, w, lab, mval), core_ids=list(range(8)))

    logits = np.empty((B, C), dtype=np.float32)
    cosine = np.empty((B, C), dtype=np.float32)
    for ci in range(8):
        bi, cj = ci // NCL, ci % NCL
        b0, c0 = bi * BL, cj * CL
        logits[b0 : b0 + BL, c0 : c0 + CL] = res.results[ci]["logits"]
        cosine[b0 : b0 + BL, c0 : c0 + CL] = res.results[ci]["cosine"]
    return logits, cosine
